# revision 5
# baseline (speedup 1.0000x reference)
"""Trainium2 Bass kernel for nn_CombineInputsWithConstraints.

out = homo_mask * cnn_center_crop + (1 - homo_mask) * minmax_norm(act)
where homo_mask[b,i,w] = all_c( MIN_T <= local_std_5x5(cnn)[b,i,w,c] <= MAX_T )

Strategy (per NeuronCore, 4 images each, batch sharded over 8 cores):
 - PE computes both 5x5 box sums (sum x and 25*sum x^2) via 5 shifted
   accumulating bf16 matmuls against a banded [128,124] weight matrix
   (vertical window via the band, horizontal window via 5 rhs col shifts).
 - ACT: fp32->bf16 convert, x^2, and A^2 squares.
 - DVE: d = (25*Sxx - mid) - A^2, abs-max over channels, threshold,
   per-image min-max affine, per-channel predicated blend.
 - GPSIMD: per-image min/max folding + partition all-reduce.
 - The +2-row/+2-col center-crop realignment is done by SBUF->SBUF DMA
   (compute engines require quadrant-aligned partition starts).
"""
import sys

sys.path.insert(0, "/opt/trn_rl_repo")

from contextlib import ExitStack

import numpy as np

K5 = 5
PAD = K5 // 2
C = 3
MIN_T = 0.005
MAX_T = 0.02
# in-band  <=>  625*MIN_T^2 <= 25*boxsum(x^2) - boxsum(x)^2 <= 625*MAX_T^2
_LO = 625.0 * MIN_T * MIN_T
_HI = 625.0 * MAX_T * MAX_T
MID = (_LO + _HI) / 2.0
HWID = (_HI - _LO) / 2.0

N_CORES = 8
ABLATE = set()   # dev-only: op groups to skip when building (perf ablation)
FULL_B = 32
FULL_H = 720
FULL_W = 1280


def _geometry(Hx, Wx):
    HV, WV = Hx - 2 * PAD, Wx - 2 * PAD
    WX_F = Wx * C          # X tile free width (elems)
    WV_F = WV * C          # valid free width
    XR = min(128, Hx)      # X tile rows (matmul K)
    M = XR - 4             # out rows per tile
    T = -(-HV // M)        # tiles per image
    xs = [min(t * M, Hx - XR) for t in range(T)]
    # superchunks over WV_F: <=1020 wide, divisible by 3
    scs = []
    off = 0
    while off < WV_F:
        w = min(1020, WV_F - off)
        scs.append((off, w))
        off += w
    # matmul pieces within a superchunk: (col_in_sc, psum_col, n) with n<=510
    # psum cols bank-aligned (512 stride) so each matmul stays in one bank
    def pieces(scw):
        ps = []
        off = 0
        bank = 0
        while off < scw:
            n = min(510, scw - off)
            ps.append((off, bank * 512, n))
            off += n
            bank += 1
        return ps

    qw = -(-WV_F // 4)
    quarters = [(i * qw, min(qw, WV_F - i * qw)) for i in range(4)
                if i * qw < WV_F]
    halfpx = WV // 2
    return dict(HV=HV, WV=WV, WX_F=WX_F, WV_F=WV_F, XR=XR, M=M, T=T, xs=xs,
                scs=scs, pieces=pieces, quarters=quarters, halfpx=halfpx)


def make_bands(Hx, Wx):
    import ml_dtypes
    g = _geometry(Hx, Wx)
    XR, M = g["XR"], g["M"]
    band = np.zeros((XR, 2 * M), dtype=np.float32)
    for m in range(M):
        band[m:m + K5, m] = 1.0
        band[m:m + K5, M + m] = 25.0
    return band.astype(ml_dtypes.bfloat16)


def build_nc(Hx, Wx, B):
    import concourse.bass as bass
    import concourse.bacc as bacc
    from concourse import bass_isa, mybir, library_config
    import concourse.tile as tile

    g = _geometry(Hx, Wx)
    HV, WV, WX_F, WV_F = g["HV"], g["WV"], g["WX_F"], g["WV_F"]
    XR, M, T, xs = g["XR"], g["M"], g["T"], g["xs"]
    scs, pieces, quarters, halfpx = g["scs"], g["pieces"], g["quarters"], g["halfpx"]
    halfw = halfpx * C
    f32 = mybir.dt.float32
    bf16 = mybir.dt.bfloat16
    Alu = mybir.AluOpType

    nc = bacc.Bacc("TRN2", target_bir_lowering=False, debug=False,
                   enable_asserts=False, num_devices=1)
    cnn_d = nc.dram_tensor("cnn", [B, Hx, Wx, C], f32, kind="ExternalInput").ap()
    act_d = nc.dram_tensor("act", [B, HV, WV, C], f32, kind="ExternalInput").ap()
    bands_d = nc.dram_tensor("bands", [XR, 2 * M], bf16, kind="ExternalInput").ap()
    out_d = nc.dram_tensor("out", [B, HV, WV, C], f32, kind="ExternalOutput").ap()

    with tile.TileContext(nc) as tc:
        with ExitStack() as ctx:
            p_const = ctx.enter_context(tc.tile_pool(name="const", bufs=1))
            p_act = ctx.enter_context(tc.tile_pool(name="act", bufs=T + 1))
            p_x = ctx.enter_context(tc.tile_pool(name="x", bufs=2))
            p_xb = ctx.enter_context(tc.tile_pool(name="xb", bufs=2))
            p_xq = ctx.enter_context(tc.tile_pool(name="xq", bufs=1))
            p_u = ctx.enter_context(tc.tile_pool(name="u", bufs=2))
            p_d = ctx.enter_context(tc.tile_pool(name="d", bufs=1))
            p_dm = ctx.enter_context(tc.tile_pool(name="dm", bufs=2))
            p_msk = ctx.enter_context(tc.tile_pool(name="msk", bufs=2))
            p_out = ctx.enter_context(tc.tile_pool(name="outh", bufs=2))
            p_sm = ctx.enter_context(tc.tile_pool(name="sm", bufs=8))
            p_ps = ctx.enter_context(tc.tile_pool(name="ps", bufs=2, space="PSUM"))

            nc.gpsimd.load_library(library_config.mlp)
            bands = p_const.tile([XR, 2 * M], bf16)
            nc.sync.dma_start(out=bands, in_=bands_d)
            band1 = bands[:, 0:M]
            band25 = bands[:, M:2 * M]

            def a_load(img, st, t):
                # split 64+60: DGE spreads a DMA over k engines where k is
                # the largest divisor of the descriptor count <= 16, so a
                # single 124-row DMA lands on only 4 of 16 engines.
                a = p_act.tile([M, WV_F], f32, tag="act")
                av = a.rearrange("p (w c) -> p w c", c=C)
                nc.sync.dma_start(out=av[0:64],
                                  in_=act_d[img, xs[t]:xs[t] + 64])
                nc.sync.dma_start(out=av[64:M],
                                  in_=act_d[img, xs[t] + 64:xs[t] + M])
                st["act"].append(a)

            def a_reduce(st, t):
                if "minmax" in ABLATE:
                    return
                rmm = p_sm.tile([M, 2], f32, tag="rmm")
                nc.vector.tensor_reduce(rmm[:, 0:1], st["act"][t],
                                        axis=mybir.AxisListType.X,
                                        op=Alu.min)
                nc.vector.tensor_reduce(rmm[:, 1:2], st["act"][t],
                                        axis=mybir.AxisListType.X,
                                        op=Alu.max)
                if t == 0:
                    st["acc"] = p_sm.tile([M, 2], f32, tag="acc", name="acc")
                    nc.vector.tensor_copy(st["acc"], rmm)
                else:
                    acc = st["acc"]
                    nc.vector.tensor_tensor(acc[:, 0:1], acc[:, 0:1],
                                            rmm[:, 0:1], op=Alu.min)
                    nc.vector.tensor_tensor(acc[:, 1:2], acc[:, 1:2],
                                            rmm[:, 1:2], op=Alu.max)

            def a_final(st):
                if "minmax" in ABLATE:
                    st["s"] = st["b"] = None
                    return
                acc = st["acc"]
                # sm = [col_max, -col_min]; all-reduce max across partitions
                sm = p_sm.tile([M, 2], f32, tag="sm2")
                nc.vector.tensor_copy(sm[:, 0:1], acc[:, 1:2])
                nc.vector.tensor_scalar(sm[:, 1:2], acc[:, 0:1], -1.0, None,
                                        op0=Alu.mult)
                smr = p_sm.tile([M, 2], f32, tag="sm2")
                nc.gpsimd.partition_all_reduce(smr, sm, channels=M,
                                               reduce_op=bass_isa.ReduceOp.max)
                # smr[:,0]=gmax, smr[:,1]=-gmin on every partition
                diff = p_sm.tile([M, 1], f32, tag="sm1")
                nc.vector.tensor_add(diff, smr[:, 0:1], smr[:, 1:2])
                sbb = p_sm.tile([M, 2], f32, tag="sbb")
                nc.vector.reciprocal(sbb[:, 0:1], diff)
                nc.vector.tensor_mul(sbb[:, 1:2], smr[:, 1:2], sbb[:, 0:1])
                st["s"], st["b"] = sbb[:, 0:1], sbb[:, 1:2]

            def b_tile(img, st, t):
                    act_t, s_sc, b_sc = st["act"], st["s"], st["b"]
                    x = p_x.tile([XR, WX_F], f32, tag="x")
                    nc.sync.dma_start(
                        out=x.rearrange("p (w c) -> p w c", c=C),
                        in_=cnn_d[img, xs[t]:xs[t] + XR])
                    xb = p_xb.tile([XR, WX_F], bf16, tag="xb")
                    xq = p_xq.tile([XR, WX_F], bf16, tag="xq")
                    if "conv" not in ABLATE:
                        nc.scalar.copy(xb, x)
                        nc.scalar.square(xq, x)
                    # center-crop halves early (SBUF->SBUF partition shift)
                    ohs = []
                    for h in range(2):
                        if "pred" in ABLATE:
                            break
                        h0 = h * halfw
                        oh = p_out.tile([M, halfw], f32, tag="outh")
                        nc.sync.dma_start(
                            out=oh[0:64],
                            in_=x[2:66, 2 * C + h0:2 * C + h0 + halfw])
                        nc.sync.dma_start(
                            out=oh[64:M],
                            in_=x[66:2 + M, 2 * C + h0:2 * C + h0 + halfw])
                        ohs.append(oh)

                    dmax = p_dm.tile([M, WV], bf16, tag="dm")
                    for si, (sc0, scw) in enumerate(scs):
                        aps = p_ps.tile([M, 1024], f32, tag="aps")
                        for (poff, pcol, n) in pieces(scw):
                            for j in range(K5):
                                if "mm" in ABLATE:
                                    break
                                c0 = sc0 + poff + C * j
                                nc.tensor.matmul(
                                    aps[:, pcol:pcol + n], band1,
                                    xb[:, c0:c0 + n],
                                    start=(j == 0), stop=(j == K5 - 1))
                        u = p_u.tile([M, 1020], bf16, tag="u")
                        pcs = pieces(scw)
                        if "usq" not in ABLATE:
                            if len(pcs) == 2 and pcs[1][2] == 510:
                                nc.scalar.square(
                                    u.rearrange("p (b k) -> p b k", b=2),
                                    aps.rearrange("p (b k) -> p b k", b=2)
                                    [:, :, 0:510])
                            else:
                                for (poff, pcol, n) in pcs:
                                    nc.scalar.square(u[:, poff:poff + n],
                                                     aps[:, pcol:pcol + n])
                        qps = p_ps.tile([M, 1024], f32, tag="qps")
                        for (poff, pcol, n) in pieces(scw):
                            for j in range(K5):
                                if "mm" in ABLATE:
                                    break
                                c0 = sc0 + poff + C * j
                                nc.tensor.matmul(
                                    qps[:, pcol:pcol + n], band25,
                                    xq[:, c0:c0 + n],
                                    start=(j == 0), stop=(j == K5 - 1))
                        d = p_d.tile([M, 1020], bf16, tag="d")
                        if "dsub" not in ABLATE:
                            if len(pcs) == 2 and pcs[1][2] == 510:
                                nc.vector.scalar_tensor_tensor(
                                    out=d.rearrange("p (b k) -> p b k", b=2),
                                    in0=qps.rearrange("p (b k) -> p b k", b=2)
                                    [:, :, 0:510],
                                    scalar=-MID,
                                    in1=u.rearrange("p (b k) -> p b k", b=2),
                                    op0=Alu.add, op1=Alu.subtract)
                            else:
                                for (poff, pcol, n) in pcs:
                                    nc.vector.scalar_tensor_tensor(
                                        out=d[:, poff:poff + n],
                                        in0=qps[:, pcol:pcol + n], scalar=-MID,
                                        in1=u[:, poff:poff + n],
                                        op0=Alu.add, op1=Alu.subtract)
                        if "absred" in ABLATE:
                            continue
                        nc.vector.tensor_reduce(
                            dmax[:, sc0 // C:(sc0 + scw) // C],
                            d[:, 0:scw].rearrange("p (w c) -> p w c", c=C),
                            axis=mybir.AxisListType.X, op=Alu.max,
                            apply_absolute_value=True)
                    # homo3 = per-element mask (mask value repeated across C)
                    # so the blend below is one contiguous copy_predicated
                    # per half instead of 3 strided per-channel ones.
                    homo3 = p_msk.tile([M, WV_F], mybir.dt.uint8, tag="msk")
                    if "homo" not in ABLATE:
                        nc.vector.tensor_scalar(
                            homo3.rearrange("p (w c) -> p w c", c=C),
                            dmax[:, :].rearrange("p (w one) -> p w one",
                                                 one=1)
                            .broadcast_to([M, WV, C]),
                            HWID, None, op0=Alu.is_le)
                    # norm in place: act = Identity(act*s + b) on ACT
                    if "norm" not in ABLATE and "minmax" not in ABLATE:
                        nc.scalar.activation(
                            act_t[t], act_t[t],
                            mybir.ActivationFunctionType.Identity,
                            bias=b_sc, scale=s_sc)
                    for h, oh in enumerate(ohs):
                        h0 = h * halfw
                        nc.vector.copy_predicated(
                            act_t[t][:, h0:h0 + halfw],
                            homo3[:, h0:h0 + halfw], oh)
                    av_all = act_t[t].rearrange("p (w c) -> p w c", c=C)
                    nc.sync.dma_start(
                        out=out_d[img, xs[t]:xs[t] + 64],
                        in_=av_all[0:64])
                    nc.sync.dma_start(
                        out=out_d[img, xs[t] + 64:xs[t] + M],
                        in_=av_all[64:M])

            # image-level software pipeline interleaved at tile granularity:
            # pair t emits [next image's act load t] [this image's blend t]
            # [next image's minmax reduce t], so the prefetch DMA runs under
            # this image's compute and the reduce never stalls DVE.
            st0 = {"act": []}
            for t in range(T):
                a_load(0, st0, t)
                a_reduce(st0, t)
            a_final(st0)
            cur = st0
            for img in range(B):
                nxt = {"act": []} if img + 1 < B else None
                for t in range(T):
                    if nxt is not None:
                        a_load(img + 1, nxt, t)
                    b_tile(img, cur, t)
                    if nxt is not None:
                        a_reduce(nxt, t)
                if nxt is not None:
                    a_final(nxt)
                cur = nxt
    nc.compile()
    return nc


_CACHE = {}


def _get_nc(Hx, Wx, B):
    key = (Hx, Wx, B)
    if key not in _CACHE:
        _CACHE[key] = build_nc(Hx, Wx, B)
    return _CACHE[key]


def kernel(cnn_inputs: np.ndarray, constrained_activations: np.ndarray) -> np.ndarray:
    from concourse.bass_utils import run_bass_kernel_spmd

    B, Hx, Wx, _ = cnn_inputs.shape
    per = B // N_CORES
    nc = _get_nc(Hx, Wx, per)
    bands = make_bands(Hx, Wx)
    cnn = np.ascontiguousarray(cnn_inputs, dtype=np.float32)
    act = np.ascontiguousarray(constrained_activations, dtype=np.float32)
    in_maps = [
        {"cnn": cnn[i * per:(i + 1) * per],
         "act": act[i * per:(i + 1) * per],
         "bands": bands}
        for i in range(N_CORES)
    ]
    res = run_bass_kernel_spmd(nc, in_maps, core_ids=list(range(N_CORES)))
    return np.concatenate([r["out"] for r in res.results], axis=0)

